# revision 45
# baseline (speedup 1.0000x reference)
"""Trainium2 Bass kernel for nn_MultiHeadAttention_48086453846410.

Reference computation (heads folded into the sequence axis, softmax over the
FULL L = seq*heads key axis; note swapped wk/wq, kept faithful):
    qp = (q @ wk_w.T + wk_b).reshape(bs, L, d)
    kp = (k @ wq_w.T + wq_b).reshape(bs, L, d)
    vp = (v @ wv_w.T + wv_b).reshape(bs, L, d)
    scores = qp @ kp.T / sqrt(d); attn = softmax(scores, -1)
    out = (attn @ vp).reshape(bs, seq, d*heads) @ out_w.T + out_b

Sharding: 8 cores = (batch b in 0..3) x (seq half); softmax is over keys, so
query rows are independent -> no collectives.

vs the bf16 baseline (315us, ~95% of the bf16 PE roofline), this version is
built around fp8-e4m3 DoubleRow matmuls (0.5 cycles/out-row with a K=256
contraction = 4x bf16 MAC throughput in the cost model) with the numerics
arranged so the 2e-2 error gate holds (measured 9.5e-3):

 - q/k projections + scores: plain e4m3 (the q/k path tolerates it, ~5e-3).
 - v projection: fp8 DR with a hi+lo split (vh.wh + vh.wl + vl.wh), which is
   v-path-accurate BELOW bf16 level; vp is stored at 64x scale so operand
   pre-scales fold away (the Z chain uses ones=64.0 and +64*4096).
 - attn@v uses o*Z = colsum(vp) + sum_m r_m vp8_m with r = exp(s)-1
   (|r|~0.2): the dominant mean signal comes from an exact colsum (computed
   algebraically as (sum_t 4v) @ (16*sum_g Wv-blocks) + bias row: two DVE
   free-dim reduces + 8 tiny matmuls), and fp8 noise only rides the small
   fluctuation term.  Z = 64*(4096 + sum r) accumulates on the PE in its own
   PSUM bank; the +const and +colsum terms are rank-1 bf16 matmuls at the
   END of each PSUM chain so pairs never wait on colsum.
 - out-projection stays bf16 (o/ow at e4m3 measured 3.2e-2 alone).

Schedule: one CONTINUOUS 128-chunk scores->exp->r8 stream across all 4
l-slices.  The A phases overlap it: A1 runs first (DMA-paced), then a merged
loop emits one A2 chunk + one A3 tile + one stream chunk per iteration so
PE, Act (A2 casts + exps), DVE (A3 adds + r-subs) and Pool (vp8 copies) all
run concurrently.  Pair accumulations (po/psZ), chain tails, Z-recip + oT
normalization, and the out-projection are scheduled at fixed stream
positions ~18-28 chunks behind the scores.  PSUM: 3 banks rotate the scores
/projection tiles, 5 banks carry the merged-loop chunks then the per-slice
chains + out-projection accumulators.  All bulk DMA descriptors issue on
the sync queue (scalar-issued DMAs would steal Act time, which gates both
the merged loop and the exp stream); per-core activations ride gpsimd's
SWDGE.  Engine-cost facts from the v2 cost model that shaped this: Pool
elementwise ops run at 0.6 efficiency (806ns/512 cols), DVE SBUF-to-SBUF
gets a 2x mode (343ns), Act ops pay a ~185ns access bubble, and same-bank
PSUM accumulation chains serialize at ~400ns/link unless interleaved.

TimelineSim: 142067 ns (baseline 315452 ns, 2.22x).
"""

import math
import sys

for _p in ("/opt/trn_rl_repo",):
    if _p not in sys.path:
        sys.path.insert(0, _p)

import numpy as np
import ml_dtypes

BS, SEQ, D, HEADS = 4, 512, 512, 8
NCORES = 8
S = SEQ // 2            # 256 query seq rows per core
HD = HEADS * D          # 4096 projection dim
JT = HD // 128          # 32 tiles of the projection dim
DT = D // 128           # 4 tiles of the 512 contraction dim
NP = DT // 2            # 2 DoubleRow k-tile pairs over d
TT = SEQ // 128         # 4 key-seq tiles per head
LSLICES = 4             # l' = 2048 per core, processed in 4 slices of 512
NP_BF16 = ml_dtypes.bfloat16
NP_E4 = ml_dtypes.float8_e4m3

_CACHE = {}


def _build_program():
    from concourse import bacc
    import concourse.mybir as mybir
    import concourse.tile as tile
    from concourse.dt import dt

    f32 = dt.float32
    b16 = dt.bfloat16
    f8 = dt.float8e4
    Act = mybir.ActivationFunctionType
    DR = mybir.MatmulPerfMode.DoubleRow
    ALU = mybir.AluOpType

    nc = bacc.Bacc(None, target_bir_lowering=False, debug=False,
                   num_devices=NCORES)

    def din(name, shape, dty=b16):
        return nc.dram_tensor(name, shape, dty, kind="ExternalInput").ap()

    qk8 = din("qk8", [D, S + SEQ], f8)     # [q[b,half].T | k[b].T]   (d, s|t)
    vT8h = din("vT8h", [D, SEQ], f8)       # e4(4*v[b].T)  hi         (d, t)
    vT8l = din("vT8l", [D, SEQ], f8)       # e4(4*v.T - hi) lo        (d, t)
    wk8 = din("wk8", [D, HD], f8)          # 64*wk_w.T  e4m3          (d, j)
    wq8 = din("wq8", [D, HD], f8)          # 64*wq_w.T  e4m3          (d, j)
    wv8h = din("wv8h", [D, HD], f8)        # e4(16*wv_w.T)  hi        (d, j)
    wv8l = din("wv8l", [D, HD], f8)        # e4(16*wv.T - hi) lo      (d, j)
    wvs16 = din("wvs16", [D, D])           # 16*sum_g wv.T blocks bf16 (d, e)
    owT = din("owT", [HD, D])              # out_w.T  bf16            (c, r)
    pbias = din("pbias", [128, 2 * JT], f32)   # [0:32]=wk_b2T [32:64]=wq_b2T
    out_br = din("out_br", [128, D], f32)      # out_b replicated
    wvbr64 = din("wvbr64", [128, HD])          # 64*wv_b replicated, bf16
    ones8 = din("ones8", [128, 256], f8)    # 1.0s: Z DoubleRow lhsT
    consts = din("consts", [128, 1152])     # 1.0 | 64*4096 | colsum bias row
    out = nc.dram_tensor("out", [S, D], f32, kind="ExternalOutput").ap()

    inv_sqrt_d = 1.0 / math.sqrt(D)

    with tile.TileContext(nc) as tc:
        with (
            tc.tile_pool(name="const", bufs=1) as cp,
            tc.tile_pool(name="wqk", bufs=8) as wp,
            tc.tile_pool(name="wvp", bufs=8) as wvp,
            tc.tile_pool(name="acts", bufs=1) as acp,
            tc.tile_pool(name="state", bufs=1) as sp,
            tc.tile_pool(name="rpairs", bufs=14) as ep,
            tc.tile_pool(name="exs", bufs=6) as xp,
            tc.tile_pool(name="zrp", bufs=2) as zp,
            tc.tile_pool(name="owp", bufs=24) as owp,
            tc.tile_pool(name="psA", bufs=3, space="PSUM") as psA,
            tc.tile_pool(name="psO", bufs=5, space="PSUM") as psO,
        ):
            # ---- fp8 weight streaming: DoubleRow pair tiles ----
            # tile (p, wq) holds d rows [p*256, (p+1)*256) as [128, 2, 1024]:
            # partition = d within 128-block, dim1 = the two d-blocks of the
            # DR pair, dim2 = j window.  Emission order = consumption order.
            def load_w8(dram_t, nm, engines):
                tiles = {}
                i = 0
                for wq_i in range(4):
                    for p in range(NP):
                        t = wp.tile([128, 2048], f8, tag="w",
                                    name=f"w8_{nm}_{p}_{wq_i}")
                        engines[i % len(engines)].dma_start(
                            out=t.rearrange("p (two j) -> p two j", two=2),
                            in_=dram_t[p * 256:(p + 1) * 256,
                                       wq_i * 1024:(wq_i + 1) * 1024]
                            .rearrange("(two p) j -> p two j", p=128))
                        tiles[(p, wq_i)] = t
                        i += 1
                return tiles

            def w8slice(tiles, p, j0, tw=1024):
                wq_i, off = divmod(j0, tw)
                return tiles[(p, wq_i)].rearrange(
                    "p (two j) -> p two j", two=2)[:, :, off:off + 128]

            # bf16 quarter-tile streaming for wv (baseline scheme)
            def load_w16(dram_t, nm, engines):
                tiles = {}
                i = 0
                for wq_i in range(4):
                    for dt_ in range(DT):
                        t = wp.tile([128, 1024], b16, tag="w",
                                    name=f"w_{nm}_{dt_}_{wq_i}")
                        engines[i % len(engines)].dma_start(
                            out=t,
                            in_=dram_t[dt_ * 128:(dt_ + 1) * 128,
                                       wq_i * 1024:(wq_i + 1) * 1024])
                        tiles[(dt_, wq_i)] = t
                        i += 1
                return tiles

            def w16slice(tiles, dt_, j0, width):
                hf, off = divmod(j0, 2048)
                return tiles[(dt_, hf)][:, off:off + width]

            # phase-A1 critical path first: q/k (small) then wk weights
            qk8_sb = acp.tile([128, DT * (S + SEQ)], f8, tag="qk")
            nc.sync.dma_start(
                out=qk8_sb.rearrange("p (t n) -> p t n", n=S + SEQ),
                in_=qk8.rearrange("(t p) n -> p t n", p=128))
            pbias_sb = cp.tile([128, 2 * JT], f32, tag="pbias")
            nc.sync.dma_start(out=pbias_sb, in_=pbias)
            wk_b2T_sb = pbias_sb[:, 0:JT]
            wq_b2T_sb = pbias_sb[:, JT:2 * JT]
            wk_sb = load_w8(wk8, "k", [nc.sync, nc.scalar])
            vT8h_sb = acp.tile([128, DT * SEQ], f8, tag="vTh")
            nc.gpsimd.dma_start(out=vT8h_sb.rearrange("p (t n) -> p t n", n=SEQ),
                                in_=vT8h.rearrange("(t p) n -> p t n", p=128))
            vT8l_sb = acp.tile([128, DT * SEQ], f8, tag="vTl")
            nc.gpsimd.dma_start(out=vT8l_sb.rearrange("p (t n) -> p t n", n=SEQ),
                                in_=vT8l.rearrange("(t p) n -> p t n", p=128))

            consts_sb = cp.tile([128, 1152], b16, tag="consts")
            nc.sync.dma_start(out=consts_sb, in_=consts)
            wvs16_sb = cp.tile([128, DT * D], b16, tag="wvs")
            nc.sync.dma_start(out=wvs16_sb.rearrange("p (t n) -> p t n", n=D),
                              in_=wvs16.rearrange("(t p) n -> p t n", p=128))

            # ---- persistent state ----
            # qpT8 interleaved: col block (dt*HEADS + h)*S, stored as 2*qp
            qpT8_sb = sp.tile([128, JT * S], f8, tag="qpT")       # 8KB/part
            kpT8_sb = sp.tile([128, JT * SEQ], f8, tag="kpT")     # 16KB/part
            vp_sb = sp.tile([128, TT * HD], b16, tag="vp")        # 32KB/part
            vp8_sb = sp.tile([128, TT * HD], f8, tag="vp8")       # 16KB/part
            oT_sb = sp.tile([128, DT * 2048], b16, tag="oT")      # 16KB/part
            fin32 = sp.tile([128, 2 * D], f32, tag="fin32")       # 4KB/part
            colrow_sb = sp.tile([1, 512], b16, tag="colrow")

            qkv_ = qk8_sb.rearrange("p (t n) -> p t n", n=S + SEQ)
            qview = qkv_[:, :, 0:S]
            kview_in = qkv_[:, :, S:S + SEQ]

            # ---- phase A1: qpT8[j, s] = 2*(wk.T @ q + wk_b), fp8 DR ----
            for jt in range(JT):
                h, dt_of_j = divmod(jt, DT)
                ps = psA.tile([128, 512], f32, tag="psA")
                for p in range(NP):
                    nc.tensor.matmul(
                        ps[:, :S],
                        lhsT=w8slice(wk_sb, p, jt * 128),
                        rhs=qview[:, 2 * p:2 * p + 2, :],
                        start=(p == 0), stop=(p == NP - 1), perf_mode=DR)
                blk = dt_of_j * HEADS + h
                # PSUM->fp8 cast with bias: only DVE/Act can read PSUM
                if jt % 2 == 0:
                    nc.vector.tensor_scalar(
                        qpT8_sb[:, blk * S:(blk + 1) * S], ps[:, :S],
                        1.0 / 32.0, wk_b2T_sb[:, jt:jt + 1],
                        op0=ALU.mult, op1=ALU.add)
                else:
                    nc.scalar.activation(
                        qpT8_sb[:, blk * S:(blk + 1) * S], ps[:, :S],
                        Act.Identity, bias=wk_b2T_sb[:, jt:jt + 1],
                        scale=1.0 / 32.0)

            # ---- fused main stream ------------------------------------
            # A2 (kp8 projection), A3 (vp projection), colsum, and the FULL
            # 128-chunk scores/exp/sub stream of all 4 l-slices run as ONE
            # continuous pipeline: the B chunk stream starts while A2/A3 are
            # still in flight (chunk (ls=0, ci) only needs A2 casts jt <=
            # 4*(ci//4)+3, emitted 4+ iters earlier), and never pauses at
            # slice boundaries, keeping the Act engine (the exp throughput
            # floor) saturated.  Pair accumulations into psZ/po, chain tails
            # (the +4096 and +colsum rank-1s, moved to the END of each PSUM
            # chain so pairs never wait on colsum), epilogues (Z-recip + oT
            # normalize) and out-projections are scheduled at fixed stream
            # positions ~LAG chunks behind the scores.
            # A2: kpT8[j, t] = 2*(wq.T @ k + wq_b), fp8 DR   (cast on Act)
            # A3: vp[t, j] = v.T @ wv + wv_b, bf16           (add on DVE)

            # weight stream in consumption order: block w holds wq8 pair
            # tiles (:, w) and wv quarter tiles (:, w); with A3 iterated
            # js-major (js = i//4), block w is first needed at iter 8w.
            # all stream DMAs on sync: scalar-issued descriptor-gen holds
            # the Act engine ~0.7-1us each, and Act gates the merged loop
            wq_sb, wv_sb = {}, {}
            eng = [nc.sync, nc.sync]
            ei = 0
            for w in range(4):
                for p in range(NP):
                    t = wp.tile([128, 2048], f8, tag="w",
                                name=f"w8_q_{p}_{w}")
                    eng[ei % 2].dma_start(
                        out=t.rearrange("p (two j) -> p two j", two=2),
                        in_=wq8[p * 256:(p + 1) * 256,
                                w * 1024:(w + 1) * 1024]
                        .rearrange("(two p) j -> p two j", p=128))
                    wq_sb[(p, w)] = t
                    ei += 1
                for hl, wsrc in (("h", wv8h), ("l", wv8l)):
                    for p in range(NP):
                        t = wvp.tile([128, 2048], f8, tag="wv",
                                     name=f"w_v{hl}_{p}_{w}")
                        eng[ei % 2].dma_start(
                            out=t.rearrange("p (two j) -> p two j", two=2),
                            in_=wsrc[p * 256:(p + 1) * 256,
                                     w * 1024:(w + 1) * 1024]
                            .rearrange("(two p) j -> p two j", p=128))
                        wv_sb[(hl, p, w)] = t
                        ei += 1
                if w == 0:
                    wv_br_sb = cp.tile([128, HD], b16, tag="wvb")
                    nc.sync.dma_start(out=wv_br_sb, in_=wvbr64)
                    ones8_sb = cp.tile([128, 256], f8, tag="ones8")
                    nc.sync.dma_start(out=ones8_sb, in_=ones8)
                if w == 1:
                    out_br_sb = cp.tile([128, D], f32, tag="outb")
                    nc.sync.dma_start(out=out_br_sb, in_=out_br)

            def wvslice(tiles, hl, p, j0):
                w, off = divmod(j0, 1024)
                return tiles[(hl, p, w)].rearrange(
                    "p (two j) -> p two j", two=2)[:, :, off:off + 512]

            kview = kpT8_sb.rearrange("p (j t) -> p j t", t=SEQ)
            qpview = qpT8_sb.rearrange("p (d hs) -> p d hs", hs=HEADS * S)
            vhview = vT8h_sb.rearrange("p (t n) -> p t n", n=SEQ)
            vlview = vT8l_sb.rearrange("p (t n) -> p t n", n=SEQ)
            vview = vp8_sb.rearrange("p (t j) -> p t j", j=HD)
            oview = ones8_sb.rearrange("p (two j) -> p two j", two=2)
            onesrow = consts_sb[0:1, 0:512]
            zconst = consts_sb[0:1, 512:640]

            LAG = 18
            NPAIR = HEADS * TT // 2  # 16
            schedule = {}

            def at(pos, fn):
                schedule.setdefault(pos, []).append(fn)

            sl = [dict(psZ=None, po=None, rps={}, ow={}) for _ in range(4)]

            def make_pair(ls, k):
                def f():
                    st = sl[ls]
                    if k == 0:
                        st["psZ"] = psO.tile([128, 512], f32, tag="psO",
                                             name=f"psZ{ls}")
                        st["po"] = [psO.tile([128, 512], f32, tag="psO",
                                             name=f"po{ls}_{e}")
                                    for e in range(DT)]
                    g, tt0 = k // 2, (k % 2) * 2
                    rv = st["rps"][k].rearrange("p (two l) -> p two l", two=2)
                    nc.tensor.matmul(st["psZ"], lhsT=oview, rhs=rv,
                                     start=(k == 0), stop=False, perf_mode=DR)
                    for et in range(DT):
                        nc.tensor.matmul(
                            st["po"][et],
                            lhsT=vview[:, tt0:tt0 + 2,
                                       g * 512 + et * 128:
                                       g * 512 + (et + 1) * 128],
                            rhs=rv, start=(k == 0), stop=False, perf_mode=DR)
                return f

            def make_tails(ls):
                def f():
                    st = sl[ls]
                    nc.tensor.matmul(st["psZ"], lhsT=zconst, rhs=onesrow,
                                     start=False, stop=True)
                    for et in range(DT):
                        nc.tensor.matmul(
                            st["po"][et],
                            lhsT=colrow_sb[0:1, et * 128:(et + 1) * 128],
                            rhs=onesrow, start=False, stop=True)
                return f

            def make_epilogue(ls):
                def f():
                    st = sl[ls]
                    zr = zp.tile([128, 512], f32, tag="zr", name=f"zr{ls}")
                    nc.vector.reciprocal(zr, st["psZ"])
                    for half in range(2):
                        for et in range(DT):
                            c0 = et * 2048 + ls * 512 + half * 256
                            nc.vector.tensor_mul(
                                oT_sb[:, c0:c0 + 256],
                                st["po"][et][:, half * 256:(half + 1) * 256],
                                zr[:, half * 256:(half + 1) * 256])
                return f

            def make_prefetch_ow(ls):
                def f():
                    st = sl[ls]
                    for ct in range(2 * ls * DT, (2 * ls + 2) * DT):
                        owt = owp.tile([128, D], b16, tag="ow",
                                       name=f"ow{ct}")
                        nc.sync.dma_start(out=owt,
                                          in_=owT[ct * 128:(ct + 1) * 128, :])
                        st["ow"][ct] = owt
                return f

            def make_outproj(ls):
                def f():
                    st = sl[ls]
                    h0 = 2 * ls
                    pscs = [psO.tile([128, 512], f32, tag="psO",
                                     name=f"psc{ls}_{stq}")
                            for stq in range(2)]
                    for ci2, ct in enumerate(range(h0 * DT, (h0 + 2) * DT)):
                        h, et = divmod(ct, DT)
                        for stq in range(2):
                            nc.tensor.matmul(
                                pscs[stq],
                                lhsT=oT_sb[:, et * 2048 + h * S + stq * 128:
                                           et * 2048 + h * S +
                                           (stq + 1) * 128],
                                rhs=st["ow"][ct],
                                start=(ci2 == 0), stop=(ci2 == 2 * DT - 1))
                    for stq in range(2):
                        if ls == 0:
                            nc.vector.tensor_add(
                                fin32[:, stq * D:(stq + 1) * D],
                                pscs[stq], out_br_sb)
                        else:
                            nc.vector.tensor_add(
                                fin32[:, stq * D:(stq + 1) * D],
                                pscs[stq], fin32[:, stq * D:(stq + 1) * D])
                return f

            for ls in range(LSLICES):
                base = ls * 32
                at(base + 8 if ls < 2 else base - 16, make_prefetch_ow(ls))
                # slice 0's chains start only after the merged loop (pos 28)
                # so its psZ/po banks don't coexist with the A-phase PSUM
                # rotation; slice 1 lags 22 so its chain allocations clear
                # the DVE burst of slice 0's late epilogue; 18 otherwise
                lag = {0: 28, 1: 22, 2: 20}.get(ls, LAG)
                for k in range(NPAIR):
                    at(base + lag + k, make_pair(ls, k))
                at(base + lag + NPAIR, make_tails(ls))
                at(base + lag + NPAIR + 2, make_epilogue(ls))
                at(base + lag + NPAIR + 4, make_outproj(ls))

            def emit_chunk(C):
                ls, ci = divmod(C, 32)
                h0 = 2 * ls
                g, tt = divmod(ci, TT)
                st = sl[ls]
                # merged-region chunks (C < 28) borrow psO's banks: its
                # chains only start at pos 28, and this decouples the
                # ps2/ps3 rotation on psA from the exp-consumer latency
                pool = psO if C < 28 else psA
                ps = pool.tile([128, 512], f32,
                               tag="psO" if C < 28 else "psA",
                               name=f"psB_{C}")
                for p in range(NP):
                    nc.tensor.matmul(
                        ps,
                        lhsT=kview[:, g * DT + 2 * p:g * DT + 2 * p + 2,
                                   tt * 128:(tt + 1) * 128],
                        rhs=qpview[:, 2 * p:2 * p + 2, h0 * S:(h0 + 2) * S],
                        start=(p == 0), stop=(p == NP - 1), perf_mode=DR)
                ext = xp.tile([128, 512], b16, tag="ex", name=f"ex{C}")
                nc.scalar.activation(ext, ps, Act.Exp, bias=0.0,
                                     scale=inv_sqrt_d / 4.0)
                if tt % 2 == 0:
                    st["rps"][ci // 2] = ep.tile([128, 1024], f8, tag="rp",
                                                 name=f"rp{ls}_{ci}")
                nc.vector.tensor_scalar_sub(
                    st["rps"][ci // 2][:, (tt % 2) * 512:(tt % 2) * 512 + 512],
                    ext, 1.0)

            def run_pos(C):
                emit_chunk(C)
                for fn in schedule.pop(C, []):
                    fn()

            _cs = [None]

            def emit_colsum_reduce():
                # colsum via  colrow64 = (sum_t 4v) @ (16*sum_g Wv) +
                # 64*512*sum_g wv_b: two DVE free-dim reduces + 8 tiny
                # matmuls instead of a 32-matmul PSUM chain over vp.
                vbar_h32 = cp.tile([128, DT], f32, tag="vbh32",
                                   name="vbar_h32")
                vbar_l32 = cp.tile([128, DT], f32, tag="vbl32",
                                   name="vbar_l32")
                nc.vector.tensor_reduce(vbar_h32, vhview,
                                        mybir.AxisListType.X, ALU.add)
                nc.vector.tensor_reduce(vbar_l32, vlview,
                                        mybir.AxisListType.X, ALU.add)
                vbar_h = cp.tile([128, DT], b16, tag="vbh", name="vbar_h")
                vbar_l = cp.tile([128, DT], b16, tag="vbl", name="vbar_l")
                nc.vector.tensor_copy(vbar_h, vbar_h32)
                nc.vector.tensor_copy(vbar_l, vbar_l32)
                _cs[0] = (vbar_h, vbar_l)

            def emit_colsum_mms():
                vbar_h, vbar_l = _cs[0]
                pscol = psO.tile([1, 512], f32, tag="psO", name="pscol")
                mi = 0
                for vb in (vbar_h, vbar_l):
                    for dt_ in range(DT):
                        nc.tensor.matmul(
                            pscol, lhsT=vb[:, dt_:dt_ + 1],
                            rhs=wvs16_sb[:, dt_ * D:(dt_ + 1) * D],
                            start=(mi == 0), stop=(mi == 2 * DT - 1))
                        mi += 1
                nc.vector.tensor_add(colrow_sb[0:1, :], pscol,
                                     consts_sb[0:1, 640:1152])

            # merged A2 + A3 loop, stream positions C = i - 4
            for i in range(JT):
                # A2 chunk jt=i
                ps2 = psA.tile([128, 512], f32, tag="psA", name=f"psA2_{i}")
                for p in range(NP):
                    nc.tensor.matmul(
                        ps2,
                        lhsT=w8slice(wq_sb, p, i * 128, tw=1024),
                        rhs=kview_in[:, 2 * p:2 * p + 2, :],
                        start=(p == 0), stop=(p == NP - 1), perf_mode=DR)
                # 4:1 Act:DVE split — the merged loop is gated by Act
                # (cast 612 + exp 597 per iter); DVE has ~0.3us of slack
                if i % 5 == 4:
                    nc.vector.tensor_scalar(
                        kpT8_sb[:, i * SEQ:(i + 1) * SEQ], ps2,
                        1.0 / 32.0, wq_b2T_sb[:, i:i + 1],
                        op0=ALU.mult, op1=ALU.add)
                else:
                    nc.scalar.activation(
                        kpT8_sb[:, i * SEQ:(i + 1) * SEQ], ps2,
                        Act.Identity, bias=wq_b2T_sb[:, i:i + 1],
                        scale=1.0 / 32.0)
                # A3 tile, js-major so weight block w = js//2 is first
                # needed at iter 8w (matches the DMA stream).  fp8 DR hi/lo
                # split: ps3 = vh.wh + vh.wl + vl.wh = 64 * v@wv.T (the ll
                # term is O(1e-3) relative); vp is stored at 64x and the
                # scale folds into the Z chain (ones=64, const=64*4096).
                js, tt = divmod(i, TT)
                ps3 = psA.tile([128, 512], f32, tag="psA", name=f"psA3_{i}")
                terms = [(vhview, "h"), (vhview, "l"), (vlview, "h")]
                mi = 0
                for vv, whl in terms:
                    for p in range(NP):
                        nc.tensor.matmul(
                            ps3,
                            lhsT=vv[:, 2 * p:2 * p + 2, tt * 128:(tt + 1) * 128],
                            rhs=wvslice(wv_sb, whl, p, js * 512),
                            start=(mi == 0), stop=(mi == 2 * len(terms) - 1),
                            perf_mode=DR)
                        mi += 1
                c0 = tt * HD + js * 512
                nc.vector.tensor_add(vp_sb[:, c0:c0 + 512], ps3,
                                     wv_br_sb[:, js * 512:(js + 1) * 512])
                nc.gpsimd.tensor_copy(vp8_sb[:, c0:c0 + 512],
                                      vp_sb[:, c0:c0 + 512])
                if i == 10:
                    emit_colsum_reduce()
                if i == 18:
                    emit_colsum_mms()
                if i >= 4:
                    run_pos(i - 4)

            if False:
                emit_colsum()  # placeholder; real def above
            # (colsum emitted inline at iter 13)
            _dead = """  colrow64 = (sum_t 4v) @ (16*sum_g Wv)
            # + 64*512*sum_g wv_b: two DVE free-dim reduces + 8 tiny matmuls
            # instead of a 32-matmul PSUM chain over vp.
            vbar_h32 = cp.tile([128, DT], f32, tag="vbh32")
            vbar_l32 = cp.tile([128, DT], f32, tag="vbl32")
            nc.vector.tensor_reduce(vbar_h32, vhview, mybir.AxisListType.X,
                                    ALU.add)
            nc.vector.tensor_reduce(vbar_l32, vlview, mybir.AxisListType.X,
                                    ALU.add)
            vbar_h = cp.tile([128, DT], b16, tag="vbh")
            vbar_l = cp.tile([128, DT], b16, tag="vbl")
            nc.vector.tensor_copy(vbar_h, vbar_h32)
            nc.vector.tensor_copy(vbar_l, vbar_l32)
            pscol = psO.tile([1, 512], f32, tag="psO", name="pscol")
            mi = 0
            for vb in (vbar_h, vbar_l):
                for dt_ in range(DT):
                    nc.tensor.matmul(
                        pscol, lhsT=vb[:, dt_:dt_ + 1],
                        rhs=wvs16_sb[:, dt_ * D:(dt_ + 1) * D],
                        start=(mi == 0), stop=(mi == 2 * DT - 1))
                    mi += 1
            nc.vector.tensor_add(colrow_sb[0:1, :], pscol,
                                 consts_sb[0:1, 640:1152])
            """

            # continue the chunk stream across all remaining slices
            for C in range(JT - 4, LSLICES * 32):
                run_pos(C)
            # leftover scheduled work (slice 3 pairs 14/15, tails, epilogue,
            # out-projection)
            for pos in sorted(schedule):
                for fn in schedule[pos]:
                    fn()
            schedule.clear()

            nc.gpsimd.dma_start(out=out[0:128, :], in_=fin32[:, 0:D])
            nc.scalar.dma_start(out=out[128:256, :], in_=fin32[:, D:2 * D])

    nc.compile()
    return nc


def _get_program():
    if "nc" not in _CACHE:
        _CACHE["nc"] = _build_program()
    return _CACHE["nc"]


def _prep_shared(inputs):
    bf = NP_BF16
    e4 = NP_E4
    f32c = np.ascontiguousarray
    consts = np.ones((128, 1152), np.float32)
    consts[:, 512:640] = 64.0 * 4096.0
    shared = {
        "wk8": f32c(np.asarray(inputs["wk_w"], np.float32).T * 64).astype(e4),
        "wq8": f32c(np.asarray(inputs["wq_w"], np.float32).T * 64).astype(e4),
        "wv8h": None,  # filled below
        "wv8l": None,
        "owT": f32c(np.asarray(inputs["out_w"], np.float32).T).astype(bf),
        "pbias": np.concatenate([
            (2 * np.asarray(inputs["wk_b"], np.float32)).reshape(JT, 128).T,
            (2 * np.asarray(inputs["wq_b"], np.float32)).reshape(JT, 128).T,
        ], axis=1).copy(),
        "out_br": f32c(np.broadcast_to(
            np.asarray(inputs["out_b"], np.float32)[None, :], (128, D))),
        "wvbr64": np.broadcast_to(
            64 * np.asarray(inputs["wv_b"], np.float32)[None, :],
            (128, HD)).astype(bf).copy(),
        "ones8": np.full((128, 256), 64.0, e4),
        "consts": consts.astype(bf),
    }
    wvT = np.ascontiguousarray(np.asarray(inputs["wv_w"], np.float32).T)
    wh = (16 * wvT).astype(e4)
    shared["wv8h"] = wh
    shared["wv8l"] = (16 * wvT - wh.astype(np.float32)).astype(e4)
    shared["wvs16"] = np.ascontiguousarray(
        (16 * wvT).reshape(D, HEADS, D).sum(axis=1)).astype(bf)
    biasrow = 64.0 * SEQ * np.asarray(
        inputs["wv_b"], np.float32).reshape(HEADS, D).sum(axis=0)
    consts2 = shared["consts"].astype(np.float32)
    consts2[:, 640:1152] = biasrow[None, :]
    shared["consts"] = consts2.astype(bf)
    return shared


def _make_in_maps(inputs):
    bf = NP_BF16
    e4 = NP_E4
    shared = _prep_shared(inputs)
    q = np.asarray(inputs["q"], np.float32)
    k = np.asarray(inputs["k"], np.float32)
    v = np.asarray(inputs["v"], np.float32)
    in_maps = []
    for core in range(NCORES):
        b, half = divmod(core, 2)
        m = dict(shared)
        m["qk8"] = np.ascontiguousarray(np.concatenate(
            [q[b, half * S:(half + 1) * S, :].T, k[b].T], axis=1)).astype(e4)
        vt = np.ascontiguousarray(4 * v[b].T)
        vh = vt.astype(e4)
        m["vT8h"] = vh
        m["vT8l"] = (vt - vh.astype(np.float32)).astype(e4)
        in_maps.append(m)
    return in_maps


def kernel(**inputs):
    from concourse.bass_utils import run_bass_kernel_spmd

    nc = _get_program()
    in_maps = _make_in_maps(inputs)
    res = run_bass_kernel_spmd(nc, in_maps, core_ids=list(range(NCORES)))
    _CACHE["last_results"] = res
    out = np.empty((BS, SEQ, D), np.float32)
    for core in range(NCORES):
        b, half = divmod(core, 2)
        out[b, half * S:(half + 1) * S, :] = res.results[core]["out"]
    return out


if __name__ == "__main__":
    rng = np.random.default_rng(0)
    fake = {
        "q": rng.standard_normal((BS, SEQ, D)).astype(np.float32),
        "k": rng.standard_normal((BS, SEQ, D)).astype(np.float32),
        "v": rng.standard_normal((BS, SEQ, D)).astype(np.float32),
        "wq_w": (rng.standard_normal((D * HEADS, D)) * 0.02).astype(np.float32),
        "wq_b": (rng.standard_normal((D * HEADS,)) * 0.02).astype(np.float32),
        "wk_w": (rng.standard_normal((D * HEADS, D)) * 0.02).astype(np.float32),
        "wk_b": (rng.standard_normal((D * HEADS,)) * 0.02).astype(np.float32),
        "wv_w": (rng.standard_normal((D * HEADS, D)) * 0.02).astype(np.float32),
        "wv_b": (rng.standard_normal((D * HEADS,)) * 0.02).astype(np.float32),
        "out_w": (rng.standard_normal((D, D * HEADS)) * 0.02).astype(np.float32),
        "out_b": (rng.standard_normal((D,)) * 0.02).astype(np.float32),
    }
    o = kernel(**fake)
    print("kernel ran, out shape", o.shape, "std", o.std())


# revision 46
# speedup vs baseline: 1.0059x; 1.0059x over previous
"""Trainium2 Bass kernel for nn_MultiHeadAttention_48086453846410.

Reference computation (heads folded into the sequence axis, softmax over the
FULL L = seq*heads key axis; note swapped wk/wq, kept faithful):
    qp = (q @ wk_w.T + wk_b).reshape(bs, L, d)
    kp = (k @ wq_w.T + wq_b).reshape(bs, L, d)
    vp = (v @ wv_w.T + wv_b).reshape(bs, L, d)
    scores = qp @ kp.T / sqrt(d); attn = softmax(scores, -1)
    out = (attn @ vp).reshape(bs, seq, d*heads) @ out_w.T + out_b

Sharding: 8 cores = (batch b in 0..3) x (seq half); softmax is over keys, so
query rows are independent -> no collectives.

vs the bf16 baseline (315us, ~95% of the bf16 PE roofline), this version is
built around fp8-e4m3 DoubleRow matmuls (0.5 cycles/out-row with a K=256
contraction = 4x bf16 MAC throughput in the cost model) with the numerics
arranged so the 2e-2 error gate holds (measured 9.5e-3):

 - q/k projections + scores: plain e4m3 (the q/k path tolerates it, ~5e-3).
 - v projection: fp8 DR with a hi+lo split (vh.wh + vh.wl + vl.wh), which is
   v-path-accurate BELOW bf16 level; vp is stored at 64x scale so operand
   pre-scales fold away (the Z chain uses ones=64.0 and +64*4096).
 - attn@v uses o*Z = colsum(vp) + sum_m r_m vp8_m with r = exp(s)-1
   (|r|~0.2): the dominant mean signal comes from an exact colsum (computed
   algebraically as (sum_t 4v) @ (16*sum_g Wv-blocks) + bias row: two DVE
   free-dim reduces + 8 tiny matmuls), and fp8 noise only rides the small
   fluctuation term.  Z = 64*(4096 + sum r) accumulates on the PE in its own
   PSUM bank; the +const and +colsum terms are rank-1 bf16 matmuls at the
   END of each PSUM chain so pairs never wait on colsum.
 - out-projection stays bf16 (o/ow at e4m3 measured 3.2e-2 alone).

Schedule: one CONTINUOUS 128-chunk scores->exp->r8 stream across all 4
l-slices.  The A phases overlap it: A1 runs first (DMA-paced), then a merged
loop emits one A2 chunk + one A3 tile + one stream chunk per iteration so
PE, Act (A2 casts + exps), DVE (A3 adds + r-subs) and Pool (vp8 copies) all
run concurrently.  Pair accumulations (po/psZ), chain tails, Z-recip + oT
normalization, and the out-projection are scheduled at fixed stream
positions ~18-28 chunks behind the scores.  PSUM: 3 banks rotate the scores
/projection tiles, 5 banks carry the merged-loop chunks then the per-slice
chains + out-projection accumulators.  All bulk DMA descriptors issue on
the sync queue (scalar-issued DMAs would steal Act time, which gates both
the merged loop and the exp stream); per-core activations ride gpsimd's
SWDGE.  Engine-cost facts from the v2 cost model that shaped this: Pool
elementwise ops run at 0.6 efficiency (806ns/512 cols), DVE SBUF-to-SBUF
gets a 2x mode (343ns), Act ops pay a ~185ns access bubble, and same-bank
PSUM accumulation chains serialize at ~400ns/link unless interleaved.

TimelineSim: 142067 ns (baseline 315452 ns, 2.22x).
"""

import math
import sys

for _p in ("/opt/trn_rl_repo",):
    if _p not in sys.path:
        sys.path.insert(0, _p)

import numpy as np
import ml_dtypes

BS, SEQ, D, HEADS = 4, 512, 512, 8
NCORES = 8
S = SEQ // 2            # 256 query seq rows per core
HD = HEADS * D          # 4096 projection dim
JT = HD // 128          # 32 tiles of the projection dim
DT = D // 128           # 4 tiles of the 512 contraction dim
NP = DT // 2            # 2 DoubleRow k-tile pairs over d
TT = SEQ // 128         # 4 key-seq tiles per head
LSLICES = 4             # l' = 2048 per core, processed in 4 slices of 512
NP_BF16 = ml_dtypes.bfloat16
NP_E4 = ml_dtypes.float8_e4m3

_CACHE = {}


def _build_program():
    from concourse import bacc
    import concourse.mybir as mybir
    import concourse.tile as tile
    from concourse.dt import dt

    f32 = dt.float32
    b16 = dt.bfloat16
    f8 = dt.float8e4
    Act = mybir.ActivationFunctionType
    DR = mybir.MatmulPerfMode.DoubleRow
    ALU = mybir.AluOpType

    nc = bacc.Bacc(None, target_bir_lowering=False, debug=False,
                   num_devices=NCORES)

    def din(name, shape, dty=b16):
        return nc.dram_tensor(name, shape, dty, kind="ExternalInput").ap()

    qT8 = din("qT8", [D, S], f8)           # q[b, half].T             (d, s)
    kT8 = din("kT8", [D, SEQ], f8)         # k[b].T                   (d, t)
    vT8h = din("vT8h", [D, SEQ], f8)       # e4(4*v[b].T)  hi         (d, t)
    vT8l = din("vT8l", [D, SEQ], f8)       # e4(4*v.T - hi) lo        (d, t)
    wk8 = din("wk8", [D, HD], f8)          # 64*wk_w.T  e4m3          (d, j)
    wq8 = din("wq8", [D, HD], f8)          # 64*wq_w.T  e4m3          (d, j)
    wv8h = din("wv8h", [D, HD], f8)        # e4(16*wv_w.T)  hi        (d, j)
    wv8l = din("wv8l", [D, HD], f8)        # e4(16*wv.T - hi) lo      (d, j)
    wvs16 = din("wvs16", [D, D])           # 16*sum_g wv.T blocks bf16 (d, e)
    owT = din("owT", [HD, D])              # out_w.T  bf16            (c, r)
    pbias = din("pbias", [128, 2 * JT], f32)   # [0:32]=wk_b2T [32:64]=wq_b2T
    out_br = din("out_br", [128, D], f32)      # out_b replicated
    wvbr64 = din("wvbr64", [128, HD])          # 64*wv_b replicated, bf16
    ones8 = din("ones8", [128, 256], f8)    # 1.0s: Z DoubleRow lhsT
    consts = din("consts", [128, 1152])     # 1.0 | 64*4096 | colsum bias row
    out = nc.dram_tensor("out", [S, D], f32, kind="ExternalOutput").ap()

    inv_sqrt_d = 1.0 / math.sqrt(D)

    with tile.TileContext(nc) as tc:
        with (
            tc.tile_pool(name="const", bufs=1) as cp,
            tc.tile_pool(name="wqk", bufs=8) as wp,
            tc.tile_pool(name="wvp", bufs=8) as wvp,
            tc.tile_pool(name="acts", bufs=1) as acp,
            tc.tile_pool(name="state", bufs=1) as sp,
            tc.tile_pool(name="rpairs", bufs=14) as ep,
            tc.tile_pool(name="exs", bufs=6) as xp,
            tc.tile_pool(name="zrp", bufs=2) as zp,
            tc.tile_pool(name="owp", bufs=24) as owp,
            tc.tile_pool(name="psA", bufs=3, space="PSUM") as psA,
            tc.tile_pool(name="psO", bufs=5, space="PSUM") as psO,
        ):
            # ---- fp8 weight streaming: DoubleRow pair tiles ----
            # tile (p, wq) holds d rows [p*256, (p+1)*256) as [128, 2, 1024]:
            # partition = d within 128-block, dim1 = the two d-blocks of the
            # DR pair, dim2 = j window.  Emission order = consumption order.
            def load_w8(dram_t, nm, engines):
                tiles = {}
                i = 0
                for wq_i in range(4):
                    for p in range(NP):
                        t = wp.tile([128, 2048], f8, tag="w",
                                    name=f"w8_{nm}_{p}_{wq_i}")
                        engines[i % len(engines)].dma_start(
                            out=t.rearrange("p (two j) -> p two j", two=2),
                            in_=dram_t[p * 256:(p + 1) * 256,
                                       wq_i * 1024:(wq_i + 1) * 1024]
                            .rearrange("(two p) j -> p two j", p=128))
                        tiles[(p, wq_i)] = t
                        i += 1
                return tiles

            def w8slice(tiles, p, j0, tw=1024):
                wq_i, off = divmod(j0, tw)
                return tiles[(p, wq_i)].rearrange(
                    "p (two j) -> p two j", two=2)[:, :, off:off + 128]

            # bf16 quarter-tile streaming for wv (baseline scheme)
            def load_w16(dram_t, nm, engines):
                tiles = {}
                i = 0
                for wq_i in range(4):
                    for dt_ in range(DT):
                        t = wp.tile([128, 1024], b16, tag="w",
                                    name=f"w_{nm}_{dt_}_{wq_i}")
                        engines[i % len(engines)].dma_start(
                            out=t,
                            in_=dram_t[dt_ * 128:(dt_ + 1) * 128,
                                       wq_i * 1024:(wq_i + 1) * 1024])
                        tiles[(dt_, wq_i)] = t
                        i += 1
                return tiles

            def w16slice(tiles, dt_, j0, width):
                hf, off = divmod(j0, 2048)
                return tiles[(dt_, hf)][:, off:off + width]

            # phase-A1 critical path first: q/k (small) then wk weights
            qT8_sb = acp.tile([128, DT * S], f8, tag="qT")
            nc.sync.dma_start(
                out=qT8_sb.rearrange("p (t n) -> p t n", n=S),
                in_=qT8.rearrange("(t p) n -> p t n", p=128))
            kT8_sb = acp.tile([128, DT * SEQ], f8, tag="kT")
            nc.gpsimd.dma_start(
                out=kT8_sb.rearrange("p (t n) -> p t n", n=SEQ),
                in_=kT8.rearrange("(t p) n -> p t n", p=128))
            pbias_sb = cp.tile([128, 2 * JT], f32, tag="pbias")
            nc.sync.dma_start(out=pbias_sb, in_=pbias)
            wk_b2T_sb = pbias_sb[:, 0:JT]
            wq_b2T_sb = pbias_sb[:, JT:2 * JT]
            wk_sb = load_w8(wk8, "k", [nc.sync, nc.scalar])
            vT8h_sb = acp.tile([128, DT * SEQ], f8, tag="vTh")
            nc.gpsimd.dma_start(out=vT8h_sb.rearrange("p (t n) -> p t n", n=SEQ),
                                in_=vT8h.rearrange("(t p) n -> p t n", p=128))
            vT8l_sb = acp.tile([128, DT * SEQ], f8, tag="vTl")
            nc.gpsimd.dma_start(out=vT8l_sb.rearrange("p (t n) -> p t n", n=SEQ),
                                in_=vT8l.rearrange("(t p) n -> p t n", p=128))

            consts_sb = cp.tile([128, 1152], b16, tag="consts")
            nc.sync.dma_start(out=consts_sb, in_=consts)
            wvs16_sb = cp.tile([128, DT * D], b16, tag="wvs")
            nc.sync.dma_start(out=wvs16_sb.rearrange("p (t n) -> p t n", n=D),
                              in_=wvs16.rearrange("(t p) n -> p t n", p=128))

            # ---- persistent state ----
            # qpT8 interleaved: col block (dt*HEADS + h)*S, stored as 2*qp
            qpT8_sb = sp.tile([128, JT * S], f8, tag="qpT")       # 8KB/part
            kpT8_sb = sp.tile([128, JT * SEQ], f8, tag="kpT")     # 16KB/part
            vp_sb = sp.tile([128, TT * HD], b16, tag="vp")        # 32KB/part
            vp8_sb = sp.tile([128, TT * HD], f8, tag="vp8")       # 16KB/part
            oT_sb = sp.tile([128, DT * 2048], b16, tag="oT")      # 16KB/part
            fin32 = sp.tile([128, 2 * D], f32, tag="fin32")       # 4KB/part
            colrow_sb = sp.tile([1, 512], b16, tag="colrow")

            qview = qT8_sb.rearrange("p (t n) -> p t n", n=S)
            kview_in = kT8_sb.rearrange("p (t n) -> p t n", n=SEQ)

            # ---- phase A1: qpT8[j, s] = 2*(wk.T @ q + wk_b), fp8 DR ----
            for jt in range(JT):
                h, dt_of_j = divmod(jt, DT)
                ps = psA.tile([128, 512], f32, tag="psA")
                for p in range(NP):
                    nc.tensor.matmul(
                        ps[:, :S],
                        lhsT=w8slice(wk_sb, p, jt * 128),
                        rhs=qview[:, 2 * p:2 * p + 2, :],
                        start=(p == 0), stop=(p == NP - 1), perf_mode=DR)
                blk = dt_of_j * HEADS + h
                # PSUM->fp8 cast with bias: only DVE/Act can read PSUM
                if jt % 2 == 0:
                    nc.vector.tensor_scalar(
                        qpT8_sb[:, blk * S:(blk + 1) * S], ps[:, :S],
                        1.0 / 32.0, wk_b2T_sb[:, jt:jt + 1],
                        op0=ALU.mult, op1=ALU.add)
                else:
                    nc.scalar.activation(
                        qpT8_sb[:, blk * S:(blk + 1) * S], ps[:, :S],
                        Act.Identity, bias=wk_b2T_sb[:, jt:jt + 1],
                        scale=1.0 / 32.0)

            # ---- fused main stream ------------------------------------
            # A2 (kp8 projection), A3 (vp projection), colsum, and the FULL
            # 128-chunk scores/exp/sub stream of all 4 l-slices run as ONE
            # continuous pipeline: the B chunk stream starts while A2/A3 are
            # still in flight (chunk (ls=0, ci) only needs A2 casts jt <=
            # 4*(ci//4)+3, emitted 4+ iters earlier), and never pauses at
            # slice boundaries, keeping the Act engine (the exp throughput
            # floor) saturated.  Pair accumulations into psZ/po, chain tails
            # (the +4096 and +colsum rank-1s, moved to the END of each PSUM
            # chain so pairs never wait on colsum), epilogues (Z-recip + oT
            # normalize) and out-projections are scheduled at fixed stream
            # positions ~LAG chunks behind the scores.
            # A2: kpT8[j, t] = 2*(wq.T @ k + wq_b), fp8 DR   (cast on Act)
            # A3: vp[t, j] = v.T @ wv + wv_b, bf16           (add on DVE)

            # weight stream in consumption order: block w holds wq8 pair
            # tiles (:, w) and wv quarter tiles (:, w); with A3 iterated
            # js-major (js = i//4), block w is first needed at iter 8w.
            # all stream DMAs on sync: scalar-issued descriptor-gen holds
            # the Act engine ~0.7-1us each, and Act gates the merged loop
            wq_sb, wv_sb = {}, {}
            eng = [nc.sync, nc.sync]
            ei = 0
            for w in range(4):
                for p in range(NP):
                    t = wp.tile([128, 2048], f8, tag="w",
                                name=f"w8_q_{p}_{w}")
                    eng[ei % 2].dma_start(
                        out=t.rearrange("p (two j) -> p two j", two=2),
                        in_=wq8[p * 256:(p + 1) * 256,
                                w * 1024:(w + 1) * 1024]
                        .rearrange("(two p) j -> p two j", p=128))
                    wq_sb[(p, w)] = t
                    ei += 1
                for hl, wsrc in (("h", wv8h), ("l", wv8l)):
                    for p in range(NP):
                        t = wvp.tile([128, 2048], f8, tag="wv",
                                     name=f"w_v{hl}_{p}_{w}")
                        eng[ei % 2].dma_start(
                            out=t.rearrange("p (two j) -> p two j", two=2),
                            in_=wsrc[p * 256:(p + 1) * 256,
                                     w * 1024:(w + 1) * 1024]
                            .rearrange("(two p) j -> p two j", p=128))
                        wv_sb[(hl, p, w)] = t
                        ei += 1
                if w == 0:
                    wv_br_sb = cp.tile([128, HD], b16, tag="wvb")
                    nc.sync.dma_start(out=wv_br_sb, in_=wvbr64)
                    ones8_sb = cp.tile([128, 256], f8, tag="ones8")
                    nc.sync.dma_start(out=ones8_sb, in_=ones8)
                if w == 1:
                    out_br_sb = cp.tile([128, D], f32, tag="outb")
                    nc.sync.dma_start(out=out_br_sb, in_=out_br)

            def wvslice(tiles, hl, p, j0):
                w, off = divmod(j0, 1024)
                return tiles[(hl, p, w)].rearrange(
                    "p (two j) -> p two j", two=2)[:, :, off:off + 512]

            kview = kpT8_sb.rearrange("p (j t) -> p j t", t=SEQ)
            qpview = qpT8_sb.rearrange("p (d hs) -> p d hs", hs=HEADS * S)
            vhview = vT8h_sb.rearrange("p (t n) -> p t n", n=SEQ)
            vlview = vT8l_sb.rearrange("p (t n) -> p t n", n=SEQ)
            vview = vp8_sb.rearrange("p (t j) -> p t j", j=HD)
            oview = ones8_sb.rearrange("p (two j) -> p two j", two=2)
            onesrow = consts_sb[0:1, 0:512]
            zconst = consts_sb[0:1, 512:640]

            LAG = 18
            NPAIR = HEADS * TT // 2  # 16
            schedule = {}

            def at(pos, fn):
                schedule.setdefault(pos, []).append(fn)

            sl = [dict(psZ=None, po=None, rps={}, ow={}) for _ in range(4)]

            def make_pair(ls, k):
                def f():
                    st = sl[ls]
                    if k == 0:
                        st["psZ"] = psO.tile([128, 512], f32, tag="psO",
                                             name=f"psZ{ls}")
                        st["po"] = [psO.tile([128, 512], f32, tag="psO",
                                             name=f"po{ls}_{e}")
                                    for e in range(DT)]
                    g, tt0 = k // 2, (k % 2) * 2
                    rv = st["rps"][k].rearrange("p (two l) -> p two l", two=2)
                    nc.tensor.matmul(st["psZ"], lhsT=oview, rhs=rv,
                                     start=(k == 0), stop=False, perf_mode=DR)
                    for et in range(DT):
                        nc.tensor.matmul(
                            st["po"][et],
                            lhsT=vview[:, tt0:tt0 + 2,
                                       g * 512 + et * 128:
                                       g * 512 + (et + 1) * 128],
                            rhs=rv, start=(k == 0), stop=False, perf_mode=DR)
                return f

            def make_tails(ls):
                def f():
                    st = sl[ls]
                    nc.tensor.matmul(st["psZ"], lhsT=zconst, rhs=onesrow,
                                     start=False, stop=True)
                    for et in range(DT):
                        nc.tensor.matmul(
                            st["po"][et],
                            lhsT=colrow_sb[0:1, et * 128:(et + 1) * 128],
                            rhs=onesrow, start=False, stop=True)
                return f

            def make_epilogue(ls):
                def f():
                    st = sl[ls]
                    zr = zp.tile([128, 512], f32, tag="zr", name=f"zr{ls}")
                    nc.vector.reciprocal(zr, st["psZ"])
                    for et in range(DT):
                        c0 = et * 2048 + ls * 512
                        nc.vector.tensor_mul(oT_sb[:, c0:c0 + 512],
                                             st["po"][et], zr)
                return f

            def make_prefetch_ow(ls):
                def f():
                    st = sl[ls]
                    for ct in range(2 * ls * DT, (2 * ls + 2) * DT):
                        owt = owp.tile([128, D], b16, tag="ow",
                                       name=f"ow{ct}")
                        nc.sync.dma_start(out=owt,
                                          in_=owT[ct * 128:(ct + 1) * 128, :])
                        st["ow"][ct] = owt
                return f

            def make_outproj(ls):
                def f():
                    st = sl[ls]
                    h0 = 2 * ls
                    pscs = [psO.tile([128, 512], f32, tag="psO",
                                     name=f"psc{ls}_{stq}")
                            for stq in range(2)]
                    for ci2, ct in enumerate(range(h0 * DT, (h0 + 2) * DT)):
                        h, et = divmod(ct, DT)
                        for stq in range(2):
                            nc.tensor.matmul(
                                pscs[stq],
                                lhsT=oT_sb[:, et * 2048 + h * S + stq * 128:
                                           et * 2048 + h * S +
                                           (stq + 1) * 128],
                                rhs=st["ow"][ct],
                                start=(ci2 == 0), stop=(ci2 == 2 * DT - 1))
                    for stq in range(2):
                        if ls == 0:
                            nc.vector.tensor_add(
                                fin32[:, stq * D:(stq + 1) * D],
                                pscs[stq], out_br_sb)
                        else:
                            nc.vector.tensor_add(
                                fin32[:, stq * D:(stq + 1) * D],
                                pscs[stq], fin32[:, stq * D:(stq + 1) * D])
                return f

            for ls in range(LSLICES):
                base = ls * 32
                at(base + 8 if ls < 2 else base - 16, make_prefetch_ow(ls))
                # slice 0's chains start only after the merged loop (pos 28)
                # so its psZ/po banks don't coexist with the A-phase PSUM
                # rotation; slice 1 lags 22 so its chain allocations clear
                # the DVE burst of slice 0's late epilogue; 18 otherwise
                lag = {0: 28, 1: 22, 2: 20}.get(ls, LAG)
                for k in range(NPAIR):
                    at(base + lag + k, make_pair(ls, k))
                at(base + lag + NPAIR, make_tails(ls))
                at(base + lag + NPAIR + 1, make_epilogue(ls))
                at(base + lag + NPAIR + 4, make_outproj(ls))

            def emit_chunk(C):
                ls, ci = divmod(C, 32)
                h0 = 2 * ls
                g, tt = divmod(ci, TT)
                st = sl[ls]
                # merged-region chunks (C < 28) borrow psO's banks: its
                # chains only start at pos 28, and this decouples the
                # ps2/ps3 rotation on psA from the exp-consumer latency
                pool = psO if C < 28 else psA
                ps = pool.tile([128, 512], f32,
                               tag="psO" if C < 28 else "psA",
                               name=f"psB_{C}")
                for p in range(NP):
                    nc.tensor.matmul(
                        ps,
                        lhsT=kview[:, g * DT + 2 * p:g * DT + 2 * p + 2,
                                   tt * 128:(tt + 1) * 128],
                        rhs=qpview[:, 2 * p:2 * p + 2, h0 * S:(h0 + 2) * S],
                        start=(p == 0), stop=(p == NP - 1), perf_mode=DR)
                ext = xp.tile([128, 512], b16, tag="ex", name=f"ex{C}")
                nc.scalar.activation(ext, ps, Act.Exp, bias=0.0,
                                     scale=inv_sqrt_d / 4.0)
                if tt % 2 == 0:
                    st["rps"][ci // 2] = ep.tile([128, 1024], f8, tag="rp",
                                                 name=f"rp{ls}_{ci}")
                nc.vector.tensor_scalar_sub(
                    st["rps"][ci // 2][:, (tt % 2) * 512:(tt % 2) * 512 + 512],
                    ext, 1.0)

            def run_pos(C):
                emit_chunk(C)
                for fn in schedule.pop(C, []):
                    fn()

            _cs = [None]

            def emit_colsum_reduce():
                # colsum via  colrow64 = (sum_t 4v) @ (16*sum_g Wv) +
                # 64*512*sum_g wv_b: two DVE free-dim reduces + 8 tiny
                # matmuls instead of a 32-matmul PSUM chain over vp.
                vbar_h32 = cp.tile([128, DT], f32, tag="vbh32",
                                   name="vbar_h32")
                vbar_l32 = cp.tile([128, DT], f32, tag="vbl32",
                                   name="vbar_l32")
                nc.vector.tensor_reduce(vbar_h32, vhview,
                                        mybir.AxisListType.X, ALU.add)
                nc.vector.tensor_reduce(vbar_l32, vlview,
                                        mybir.AxisListType.X, ALU.add)
                vbar_h = cp.tile([128, DT], b16, tag="vbh", name="vbar_h")
                vbar_l = cp.tile([128, DT], b16, tag="vbl", name="vbar_l")
                nc.vector.tensor_copy(vbar_h, vbar_h32)
                nc.vector.tensor_copy(vbar_l, vbar_l32)
                _cs[0] = (vbar_h, vbar_l)

            def emit_colsum_mms():
                vbar_h, vbar_l = _cs[0]
                pscol = psO.tile([1, 512], f32, tag="psO", name="pscol")
                mi = 0
                for vb in (vbar_h, vbar_l):
                    for dt_ in range(DT):
                        nc.tensor.matmul(
                            pscol, lhsT=vb[:, dt_:dt_ + 1],
                            rhs=wvs16_sb[:, dt_ * D:(dt_ + 1) * D],
                            start=(mi == 0), stop=(mi == 2 * DT - 1))
                        mi += 1
                nc.vector.tensor_add(colrow_sb[0:1, :], pscol,
                                     consts_sb[0:1, 640:1152])

            # merged A2 + A3 loop, stream positions C = i - 4
            for i in range(JT):
                # A2 chunk jt=i
                ps2 = psA.tile([128, 512], f32, tag="psA", name=f"psA2_{i}")
                for p in range(NP):
                    nc.tensor.matmul(
                        ps2,
                        lhsT=w8slice(wq_sb, p, i * 128, tw=1024),
                        rhs=kview_in[:, 2 * p:2 * p + 2, :],
                        start=(p == 0), stop=(p == NP - 1), perf_mode=DR)
                # 4:1 Act:DVE split — the merged loop is gated by Act
                # (cast 612 + exp 597 per iter); DVE has ~0.3us of slack
                if i % 5 == 4:
                    nc.vector.tensor_scalar(
                        kpT8_sb[:, i * SEQ:(i + 1) * SEQ], ps2,
                        1.0 / 32.0, wq_b2T_sb[:, i:i + 1],
                        op0=ALU.mult, op1=ALU.add)
                else:
                    nc.scalar.activation(
                        kpT8_sb[:, i * SEQ:(i + 1) * SEQ], ps2,
                        Act.Identity, bias=wq_b2T_sb[:, i:i + 1],
                        scale=1.0 / 32.0)
                # A3 tile, js-major so weight block w = js//2 is first
                # needed at iter 8w (matches the DMA stream).  fp8 DR hi/lo
                # split: ps3 = vh.wh + vh.wl + vl.wh = 64 * v@wv.T (the ll
                # term is O(1e-3) relative); vp is stored at 64x and the
                # scale folds into the Z chain (ones=64, const=64*4096).
                js, tt = divmod(i, TT)
                ps3 = psA.tile([128, 512], f32, tag="psA", name=f"psA3_{i}")
                terms = [(vhview, "h"), (vhview, "l"), (vlview, "h")]
                mi = 0
                for vv, whl in terms:
                    for p in range(NP):
                        nc.tensor.matmul(
                            ps3,
                            lhsT=vv[:, 2 * p:2 * p + 2, tt * 128:(tt + 1) * 128],
                            rhs=wvslice(wv_sb, whl, p, js * 512),
                            start=(mi == 0), stop=(mi == 2 * len(terms) - 1),
                            perf_mode=DR)
                        mi += 1
                c0 = tt * HD + js * 512
                nc.vector.tensor_add(vp_sb[:, c0:c0 + 512], ps3,
                                     wv_br_sb[:, js * 512:(js + 1) * 512])
                nc.gpsimd.tensor_copy(vp8_sb[:, c0:c0 + 512],
                                      vp_sb[:, c0:c0 + 512])
                if i == 10:
                    emit_colsum_reduce()
                if i == 18:
                    emit_colsum_mms()
                if i >= 4:
                    run_pos(i - 4)

            if False:
                emit_colsum()  # placeholder; real def above
            # (colsum emitted inline at iter 13)
            _dead = """  colrow64 = (sum_t 4v) @ (16*sum_g Wv)
            # + 64*512*sum_g wv_b: two DVE free-dim reduces + 8 tiny matmuls
            # instead of a 32-matmul PSUM chain over vp.
            vbar_h32 = cp.tile([128, DT], f32, tag="vbh32")
            vbar_l32 = cp.tile([128, DT], f32, tag="vbl32")
            nc.vector.tensor_reduce(vbar_h32, vhview, mybir.AxisListType.X,
                                    ALU.add)
            nc.vector.tensor_reduce(vbar_l32, vlview, mybir.AxisListType.X,
                                    ALU.add)
            vbar_h = cp.tile([128, DT], b16, tag="vbh")
            vbar_l = cp.tile([128, DT], b16, tag="vbl")
            nc.vector.tensor_copy(vbar_h, vbar_h32)
            nc.vector.tensor_copy(vbar_l, vbar_l32)
            pscol = psO.tile([1, 512], f32, tag="psO", name="pscol")
            mi = 0
            for vb in (vbar_h, vbar_l):
                for dt_ in range(DT):
                    nc.tensor.matmul(
                        pscol, lhsT=vb[:, dt_:dt_ + 1],
                        rhs=wvs16_sb[:, dt_ * D:(dt_ + 1) * D],
                        start=(mi == 0), stop=(mi == 2 * DT - 1))
                    mi += 1
            nc.vector.tensor_add(colrow_sb[0:1, :], pscol,
                                 consts_sb[0:1, 640:1152])
            """

            # continue the chunk stream across all remaining slices
            for C in range(JT - 4, LSLICES * 32):
                run_pos(C)
            # leftover scheduled work (slice 3 pairs 14/15, tails, epilogue,
            # out-projection)
            for pos in sorted(schedule):
                for fn in schedule[pos]:
                    fn()
            schedule.clear()

            nc.gpsimd.dma_start(out=out[0:128, :], in_=fin32[:, 0:D])
            nc.scalar.dma_start(out=out[128:256, :], in_=fin32[:, D:2 * D])

    nc.compile()
    return nc


def _get_program():
    if "nc" not in _CACHE:
        _CACHE["nc"] = _build_program()
    return _CACHE["nc"]


def _prep_shared(inputs):
    bf = NP_BF16
    e4 = NP_E4
    f32c = np.ascontiguousarray
    consts = np.ones((128, 1152), np.float32)
    consts[:, 512:640] = 64.0 * 4096.0
    shared = {
        "wk8": f32c(np.asarray(inputs["wk_w"], np.float32).T * 64).astype(e4),
        "wq8": f32c(np.asarray(inputs["wq_w"], np.float32).T * 64).astype(e4),
        "wv8h": None,  # filled below
        "wv8l": None,
        "owT": f32c(np.asarray(inputs["out_w"], np.float32).T).astype(bf),
        "pbias": np.concatenate([
            (2 * np.asarray(inputs["wk_b"], np.float32)).reshape(JT, 128).T,
            (2 * np.asarray(inputs["wq_b"], np.float32)).reshape(JT, 128).T,
        ], axis=1).copy(),
        "out_br": f32c(np.broadcast_to(
            np.asarray(inputs["out_b"], np.float32)[None, :], (128, D))),
        "wvbr64": np.broadcast_to(
            64 * np.asarray(inputs["wv_b"], np.float32)[None, :],
            (128, HD)).astype(bf).copy(),
        "ones8": np.full((128, 256), 64.0, e4),
        "consts": consts.astype(bf),
    }
    wvT = np.ascontiguousarray(np.asarray(inputs["wv_w"], np.float32).T)
    wh = (16 * wvT).astype(e4)
    shared["wv8h"] = wh
    shared["wv8l"] = (16 * wvT - wh.astype(np.float32)).astype(e4)
    shared["wvs16"] = np.ascontiguousarray(
        (16 * wvT).reshape(D, HEADS, D).sum(axis=1)).astype(bf)
    biasrow = 64.0 * SEQ * np.asarray(
        inputs["wv_b"], np.float32).reshape(HEADS, D).sum(axis=0)
    consts2 = shared["consts"].astype(np.float32)
    consts2[:, 640:1152] = biasrow[None, :]
    shared["consts"] = consts2.astype(bf)
    return shared


def _make_in_maps(inputs):
    bf = NP_BF16
    e4 = NP_E4
    shared = _prep_shared(inputs)
    q = np.asarray(inputs["q"], np.float32)
    k = np.asarray(inputs["k"], np.float32)
    v = np.asarray(inputs["v"], np.float32)
    in_maps = []
    for core in range(NCORES):
        b, half = divmod(core, 2)
        m = dict(shared)
        m["qT8"] = np.ascontiguousarray(
            q[b, half * S:(half + 1) * S, :].T).astype(e4)
        m["kT8"] = np.ascontiguousarray(k[b].T).astype(e4)
        vt = np.ascontiguousarray(4 * v[b].T)
        vh = vt.astype(e4)
        m["vT8h"] = vh
        m["vT8l"] = (vt - vh.astype(np.float32)).astype(e4)
        in_maps.append(m)
    return in_maps


def kernel(**inputs):
    from concourse.bass_utils import run_bass_kernel_spmd

    nc = _get_program()
    in_maps = _make_in_maps(inputs)
    res = run_bass_kernel_spmd(nc, in_maps, core_ids=list(range(NCORES)))
    _CACHE["last_results"] = res
    out = np.empty((BS, SEQ, D), np.float32)
    for core in range(NCORES):
        b, half = divmod(core, 2)
        out[b, half * S:(half + 1) * S, :] = res.results[core]["out"]
    return out


if __name__ == "__main__":
    rng = np.random.default_rng(0)
    fake = {
        "q": rng.standard_normal((BS, SEQ, D)).astype(np.float32),
        "k": rng.standard_normal((BS, SEQ, D)).astype(np.float32),
        "v": rng.standard_normal((BS, SEQ, D)).astype(np.float32),
        "wq_w": (rng.standard_normal((D * HEADS, D)) * 0.02).astype(np.float32),
        "wq_b": (rng.standard_normal((D * HEADS,)) * 0.02).astype(np.float32),
        "wk_w": (rng.standard_normal((D * HEADS, D)) * 0.02).astype(np.float32),
        "wk_b": (rng.standard_normal((D * HEADS,)) * 0.02).astype(np.float32),
        "wv_w": (rng.standard_normal((D * HEADS, D)) * 0.02).astype(np.float32),
        "wv_b": (rng.standard_normal((D * HEADS,)) * 0.02).astype(np.float32),
        "out_w": (rng.standard_normal((D, D * HEADS)) * 0.02).astype(np.float32),
        "out_b": (rng.standard_normal((D,)) * 0.02).astype(np.float32),
    }
    o = kernel(**fake)
    print("kernel ran, out shape", o.shape, "std", o.std())
